# revision 8
# baseline (speedup 1.0000x reference)
"""Trainium2 Bass kernel for nn_CurveGraphic2d (retrieval_knn).

Computes, for B=16 cubic Bezier curves, a 256x256 canvas per curve:
    canvas = clip(1 - (min_dist_to_32_samples / w + eps)^aa, 0, 1)

Strategy (data-parallel over curves, 2 per core):
  * Host (numpy): evaluate the 32 Bezier sample points per curve (tiny),
    plan x-window "jobs": each job = (16-wide x-window, y-tile) holding M
    parabola rows (the samples relevant to that window).  Build psi tables
    so that the device matmul  phi^T @ psi  produces T[p, row, xl] =
    squared distance from pixel (y = ytile*128 + p, x = xoff + xl) to the
    row's sample.  Coordinates are centered to keep fp32r matmul exact
    enough.
  * Device: fp32r matmuls build T in PSUM; one 4-dim strided tensor_reduce
    per equal-M job group computes the min over rows into a packed strip;
    ACT applies ln/exp/relu tail (the pow, with sqrt folded into the
    exponent); strips DMA out.
  * Host: scatters strip columns into the zero-initialized canvas
    (pure placement; part of unsharding).

Uniform SPMD structure: all 8 cores run the same program; per-core values
ride in DMA'd tables; schedule shapes are max-padded across the curves
assigned to each of the two curve slots.
"""

import math
import os

import numpy as np

H, W = 256, 256
NUM_SAMPLES = 32
MAX_LENGTH = 300.0
EPSILON = 1e-6
N_CORES = 8
CURVES_PER_CORE = 2

WIN = 8           # x-window width (output cols per job)
MARGIN = 8.5      # sample relevance half-width (8 px + rounding slack)
PAD_SY = 1500.0   # pad-row sample y-offset (never wins the min)
BIG = 1.0e9


# ----------------------------------------------------------------------------
# Host-side geometry (mirrors reference.py in float64)
# ----------------------------------------------------------------------------

def _bezier_eval(cp, ts):
    K = cp.shape[0]
    n = K - 1
    i = np.arange(K)
    binom = np.array([math.comb(n, k) for k in range(K)], dtype=np.float64)
    t = ts[:, None]
    basis = binom * (t ** i) * ((1.0 - t) ** (n - i))
    return basis @ cp


def _decasteljau_left(cp, t):
    pts = cp.copy()
    left = [cp[0]]
    for _ in range(cp.shape[0] - 1):
        pts = (1.0 - t) * pts[:-1] + t * pts[1:]
        left.append(pts[0])
    return np.stack(left)


def compute_samples(inputs):
    """[B, K, 2] normalized control points -> [B, S, 2] sample points (y, x)."""
    ts = np.linspace(0.0, 1.0, NUM_SAMPLES)
    out = []
    for b in range(inputs.shape[0]):
        cp = inputs[b].astype(np.float64) * np.array([H, W], dtype=np.float64)
        approx = _bezier_eval(cp, ts)
        seg = np.diff(approx, axis=0)
        arc = np.sqrt((seg ** 2).sum(-1)).sum()
        t_tr = min(1.0, MAX_LENGTH / (arc + EPSILON))
        out.append(_bezier_eval(_decasteljau_left(cp, t_tr), ts))
    return np.stack(out)  # [B, S, 2] float64


# ----------------------------------------------------------------------------
# Planner: per-curve job lists, then a uniform padded schedule
# ----------------------------------------------------------------------------

class Job:
    __slots__ = ("xoff", "ytile", "rows")

    def __init__(self, xoff, ytile, rows):
        self.xoff = xoff      # int, window covers x in [xoff, xoff+WIN)
        self.ytile = ytile    # 0 or 1 (y in [ytile*128, ytile*128+128))
        self.rows = rows      # list of (sy, sx) float64 sample coords


def plan_curve(samples):
    """samples [S, 2] (y, x) -> list of Job."""
    sy = samples[:, 0]
    sx = samples[:, 1]
    # active x intervals (union of per-sample [sx-M, sx+M]), clipped to canvas
    lo = np.maximum(np.floor(sx - MARGIN).astype(int), 0)
    hi = np.minimum(np.ceil(sx + MARGIN).astype(int), W - 1)
    ivs = sorted((int(a), int(b)) for a, b in zip(lo, hi) if a <= b)
    merged = []
    for a, b in ivs:
        if merged and a <= merged[-1][1] + 1:
            merged[-1][1] = max(merged[-1][1], b)
        else:
            merged.append([a, b])
    # tile with WIN-wide windows
    xoffs = []
    for a, b in merged:
        o = a
        while o <= b:
            xoffs.append(o)
            o += WIN
    jobs = []
    for o in xoffs:
        # samples relevant in x
        selx = (sx + MARGIN >= o) & (sx - MARGIN < o + WIN)
        for yt in (0, 1):
            y0, y1 = yt * 128, yt * 128 + 128
            sely = (sy + MARGIN >= y0) & (sy - MARGIN < y1)
            sel = selx & sely
            if sel.any():
                rows = list(zip(sy[sel], sx[sel]))
                jobs.append(Job(o, yt, rows))
    return jobs


def round_up_M(m):
    for v in (2, 3, 4, 6, 8, 12, 16, 24, 32, 40, 48, 64):
        if m <= v:
            return v
    return m


def build_schedule(job_lists):
    """job_lists: per-curve list of Jobs (for the curves sharing one slot).
    Returns (schedule, padded) where schedule is a list of M values (one per
    job slot, descending), grouped so equal M's are adjacent."""
    per_curve = []
    for jl in job_lists:
        sizes = sorted((round_up_M(len(j.rows)) for j in jl), reverse=True)
        per_curve.append(sizes)
    nslots = max(len(s) for s in per_curve)
    sched = []
    for i in range(nslots):
        sched.append(max((s[i] if i < len(s) else 0) for s in per_curve))
    sched = [round_up_M(max(m, 2)) for m in sched]
    return sched


def group_schedule(sched):
    """[M descending] -> list of (g, M) groups of equal M."""
    groups = []
    for m in sched:
        if groups and groups[-1][1] == m:
            groups[-1][0] += 1
        else:
            groups.append([1, m])
    return [(g, m) for g, m in groups]


# ----------------------------------------------------------------------------
# Table building (per curve, given its slot schedule)
# ----------------------------------------------------------------------------

def q11(x):
    """Round to 11 significant bits (safely exact under fp32r's ~12-bit
    input truncation)."""
    x = np.asarray(x, dtype=np.float64)
    m, e = np.frexp(x)
    return np.ldexp(np.round(m * 2048.0), e - 11)


def build_tables(jobs, sched, width, aa):
    """Returns (psi [6, TCOLS] f32, scatter list, tcols).

    The device computes T = phi^T @ psi in fp32r (inputs truncated to
    ~12 bits).  Every psi/phi entry is built 11-bit-exact so the matmul is
    exact up to fp32 accumulation; the sample's y is effectively displaced
    by <= ~3e-5 px (sq + srq vs sy).  Features:
      phi = [q11(y'^2), y'^2 - q11(y'^2), y', y', 1, 1]     (y' = p - 64)
      psi = [1, 1, -2*sq, -2*srq, c_hi, c_lo]
    with sq = q11(sy'), srq = q11(sy' - sq), S = sq + srq,
         c = S^2 + (x'' - sx'')^2, c_hi = q11(c), c_lo = q11(c - c_hi).
    """
    tcols = sum(m * WIN for m in sched)
    psi = np.zeros((6, tcols), dtype=np.float64)
    scatter = []
    jobs_sorted = sorted(jobs, key=lambda j: len(j.rows), reverse=True)
    assert len(jobs_sorted) <= len(sched), (len(jobs_sorted), len(sched))
    col = 0
    xl = np.arange(WIN, dtype=np.float64)
    half = WIN / 2.0
    for k, M in enumerate(sched):
        job = jobs_sorted[k] if k < len(jobs_sorted) else None
        if job is not None:
            assert round_up_M(len(job.rows)) <= M
            scatter.append((job.ytile, job.xoff))
        else:
            scatter.append(None)
        for m in range(M):
            if job is not None and m < len(job.rows):
                sy, sx = job.rows[m]
                syp = sy - (job.ytile * 128 + 64.0)   # y center
                sxp = sx - (job.xoff + half)          # window-local x center
            else:
                syp = PAD_SY
                sxp = 0.0
            sq = q11(syp)
            srq = q11(syp - sq)
            S = sq + srq
            c = S * S + (xl - half - sxp) ** 2
            c_hi = q11(c)
            c_lo = q11(c - c_hi)
            c0, c1 = col + m * WIN, col + (m + 1) * WIN
            psi[0, c0:c1] = 1.0
            psi[1, c0:c1] = 1.0
            psi[2, c0:c1] = -2.0 * sq
            psi[3, c0:c1] = -2.0 * srq
            psi[4, c0:c1] = c_hi
            psi[5, c0:c1] = c_lo
        col += M * WIN
    return psi.astype(np.float32), scatter, tcols


PHI = None


def get_phi():
    global PHI
    if PHI is None:
        p = np.arange(128, dtype=np.float64) - 64.0
        y2 = p * p
        y2hi = q11(y2)
        PHI = np.stack([y2hi, y2 - y2hi, p, p,
                        np.ones(128), np.ones(128)]).astype(np.float32)
    return PHI


# ----------------------------------------------------------------------------
# Numpy simulation of the device program (for validation)
# ----------------------------------------------------------------------------

def simulate_device(psi, groups, inv_w2, half_aa):
    """Exact-semantics simulation of one curve-slot's device pipeline.
    psi [3, TCOLS]; groups [(g, M)]; returns strip [128, J*WIN] f32."""
    phi = get_phi().astype(np.float32)
    T = (phi.T.astype(np.float32) @ psi.astype(np.float32)).astype(np.float32)
    # T [128, TCOLS]; reduce per group
    strips = []
    col = 0
    for g, M in groups:
        blk = T[:, col:col + g * M * WIN].reshape(128, g, M, WIN)
        strips.append(blk.min(axis=2).reshape(128, g * WIN))
        col += g * M * WIN
    m2 = np.concatenate(strips, axis=1).astype(np.float32)
    lnv = np.log(np.maximum(m2 * inv_w2 + 1e-12, 1e-38)).astype(np.float32)
    p = np.exp(half_aa * lnv).astype(np.float32)
    return np.maximum(1.0 - p, 0.0).astype(np.float32)


def host_scatter(canvas, strip, scatter, sched):
    """Place strip columns into canvas [256, 256] for one curve."""
    col = 0
    for k, _M in enumerate(sched):
        info = scatter[k]
        if info is not None:
            yt, xoff = info
            x0 = max(xoff, 0)
            x1 = min(xoff + WIN, W)
            if x1 > x0:
                canvas[yt * 128:(yt + 1) * 128, x0:x1] = \
                    strip[:, col + (x0 - xoff):col + (x1 - xoff)]
        col += WIN
    return canvas


# ----------------------------------------------------------------------------
# Full planning for a batch of curves
# ----------------------------------------------------------------------------

class Plan:
    pass


def plan_all(inputs, widths, aas):
    """Plan the uniform schedule + per-curve tables.
    Returns a Plan with everything needed for device build + scatter."""
    B = inputs.shape[0]
    samples = compute_samples(inputs)
    jobs = [plan_curve(samples[b]) for b in range(B)]
    # assign curves to slots: sort by max job size so similar curves share
    # a slot schedule (tighter padding)
    dens = [max((len(j.rows) for j in jl), default=0) for jl in jobs]
    order = np.argsort(dens)  # ascending density
    slot_curves = [list(order[:B // 2]), list(order[B // 2:])]
    plan = Plan()
    plan.samples = samples
    plan.slot_curves = slot_curves  # slot -> list of 8 curve ids (core i gets
    #                                  slot_curves[s][i] in slot s)
    plan.scheds = []
    plan.groups = []
    plan.tcols = []
    for s in range(2):
        sched = build_schedule([jobs[c] for c in slot_curves[s]])
        plan.scheds.append(sched)
        plan.groups.append(group_schedule(sched))
        plan.tcols.append(sum(m * WIN for m in sched))
    plan.psis = {}      # curve -> psi
    plan.scatters = {}  # curve -> scatter list
    for s in range(2):
        for c in slot_curves[s]:
            psi, scatter, _ = build_tables(jobs[c], plan.scheds[s],
                                           widths[c], aas[c])
            plan.psis[c] = psi
            plan.scatters[c] = scatter
    plan.widths = widths
    plan.aas = aas
    return plan


# ----------------------------------------------------------------------------
# Bass device program
# ----------------------------------------------------------------------------

def build_bass(plan):
    import concourse.bacc as bacc
    import concourse.mybir as mybir
    from concourse.tile import TileContext

    dt = mybir.dt
    nc = bacc.Bacc(None, target_bir_lowering=False)

    tcols = plan.tcols
    scols = [len(plan.scheds[s]) * WIN for s in range(2)]

    phi_d = nc.dram_tensor("phi", [6, 128], dt.float32r, kind="ExternalInput")
    psi_d = [nc.dram_tensor(f"psi{s}", [6, tcols[s]], dt.float32r,
                            kind="ExternalInput") for s in range(2)]
    pars_d = nc.dram_tensor("pars", [128, 16], dt.float32,
                            kind="ExternalInput")
    strips_d = [nc.dram_tensor(f"strips{s}", [128, scols[s]], dt.float32,
                               kind="ExternalOutput") for s in range(2)]

    with TileContext(nc) as tc:
        with tc.tile_pool(name="sb", bufs=1) as pool, \
             tc.tile_pool(name="ps", bufs=1, space="PSUM") as ppool:
            # ACT table warm-up: issue a tiny Ln immediately so the
            # ~2.7us natural_log table load overlaps the input DMAs.
            warm = pool.tile([128, 8], dt.float32, tag="warm")
            nc.gpsimd.memset(warm[:], 1.0)
            warm2 = pool.tile([128, 8], dt.float32, tag="warm2")
            nc.scalar.activation(warm2[:], warm[:],
                                 mybir.ActivationFunctionType.Ln,
                                 bias=warm[:, 0:1], scale=1.0)

            phit = pool.tile([6, 128], dt.float32r, tag="phi")
            nc.sync.dma_start(out=phit[:], in_=phi_d[:])
            parst = pool.tile([128, 16], dt.float32, tag="pars")
            nc.sync.dma_start(out=parst[:], in_=pars_d[:])

            for s in range(2):
                TC = tcols[s]
                SC = scols[s]
                psit = pool.tile([6, TC], dt.float32r, tag=f"psi{s}")
                nc.sync.dma_start(out=psit[:], in_=psi_d[s][:])

                T = ppool.tile([128, TC], dt.float32, tag=f"T{s}")
                for c0 in range(0, TC, 512):
                    n = min(512, TC - c0)
                    nc.tensor.matmul(T[:, c0:c0 + n], phit[:],
                                     psit[:, c0:c0 + n],
                                     start=True, stop=True)

                strip = pool.tile([128, SC], dt.float32, tag=f"strip{s}")
                col = 0
                sc = 0
                for g, M in plan.groups[s]:
                    tv = T[:, col:col + g * M * WIN].rearrange(
                        "p (j m x) -> p j x m", j=g, m=M, x=WIN)
                    ov = strip[:, sc:sc + g * WIN].rearrange(
                        "p (j x) -> p j x", j=g, x=WIN)
                    nc.vector.tensor_reduce(out=ov, in_=tv,
                                            axis=mybir.AxisListType.X,
                                            op=mybir.AluOpType.min)
                    col += g * M * WIN
                    sc += g * WIN

                # tail: canvas = relu(1 - exp(aa/2 * ln(max(m2,0)/w^2 + 1e-12)))
                po = 8 * s
                nc.vector.tensor_scalar_max(strip[:], strip[:], 0.0)
                tl = pool.tile([128, SC], dt.float32, tag=f"tail{s}")
                nc.scalar.activation(tl[:], strip[:],
                                     mybir.ActivationFunctionType.Ln,
                                     bias=parst[:, po + 2:po + 3],
                                     scale=parst[:, po + 0:po + 1])
                nc.scalar.activation(tl[:], tl[:],
                                     mybir.ActivationFunctionType.Exp,
                                     bias=parst[:, po + 3:po + 4],
                                     scale=parst[:, po + 1:po + 2])
                nc.scalar.activation(tl[:], tl[:],
                                     mybir.ActivationFunctionType.Relu,
                                     bias=parst[:, po + 4:po + 5],
                                     scale=-1.0)
                nc.sync.dma_start(out=strips_d[s][:], in_=tl[:])
    nc.compile()
    return nc


def make_pars(plan, s0_curve, s1_curve):
    pars = np.zeros((128, 16), dtype=np.float32)
    for s, c in ((0, s0_curve), (1, s1_curve)):
        po = 8 * s
        pars[:, po + 0] = 1.0 / (float(plan.widths[c]) ** 2)
        pars[:, po + 1] = float(plan.aas[c]) / 2.0
        pars[:, po + 2] = 1e-12
        pars[:, po + 3] = 0.0
        pars[:, po + 4] = 1.0
    return pars


def make_in_maps(plan):
    in_maps = []
    for core in range(N_CORES):
        c0 = int(plan.slot_curves[0][core])
        c1 = int(plan.slot_curves[1][core])
        in_maps.append({
            "phi": get_phi(),
            "psi0": plan.psis[c0],
            "psi1": plan.psis[c1],
            "pars": make_pars(plan, c0, c1),
        })
    return in_maps


def scatter_all(plan, results):
    out = np.zeros((len(plan.widths), H, W), dtype=np.float32)
    for core in range(N_CORES):
        for s in range(2):
            c = int(plan.slot_curves[s][core])
            strip = results[core][f"strips{s}"]
            host_scatter(out[c], strip, plan.scatters[c], plan.scheds[s])
    return out


def kernel(inputs, widths, aa_factors):
    inputs = np.asarray(inputs, dtype=np.float32)
    widths = np.asarray(widths, dtype=np.float32)
    aa_factors = np.asarray(aa_factors, dtype=np.float32)
    plan = plan_all(inputs, widths, aa_factors)
    nc = build_bass(plan)
    from concourse.bass_utils import run_bass_kernel_spmd
    res = run_bass_kernel_spmd(nc, make_in_maps(plan),
                               core_ids=list(range(N_CORES)))
    return scatter_all(plan, res.results)


def reference_canvas_host(plan):
    """Full-pipeline numpy simulation -> [B, 256, 256] canvases."""
    B = len(plan.widths)
    out = np.zeros((B, H, W), dtype=np.float32)
    for s in range(2):
        for c in plan.slot_curves[s]:
            inv_w2 = np.float32(1.0 / (plan.widths[c] ** 2))
            half_aa = np.float32(plan.aas[c] / 2.0)
            strip = simulate_device(plan.psis[c], plan.groups[s],
                                    inv_w2, half_aa)
            host_scatter(out[c], strip, plan.scatters[c], plan.scheds[s])
    return out


# revision 13
# speedup vs baseline: 1.0585x; 1.0585x over previous
"""Trainium2 Bass kernel for nn_CurveGraphic2d (retrieval_knn).

Computes, for B=16 cubic Bezier curves, a 256x256 canvas per curve:
    canvas = clip(1 - (min_dist_to_32_samples / w + eps)^aa, 0, 1)

Strategy (data-parallel over curves, 2 per core):
  * Host (numpy): evaluate the 32 Bezier sample points per curve (tiny),
    plan x-window "jobs": each job = (16-wide x-window, y-tile) holding M
    parabola rows (the samples relevant to that window).  Build psi tables
    so that the device matmul  phi^T @ psi  produces T[p, row, xl] =
    squared distance from pixel (y = ytile*128 + p, x = xoff + xl) to the
    row's sample.  Coordinates are centered to keep fp32r matmul exact
    enough.
  * Device: fp32r matmuls build T in PSUM; one 4-dim strided tensor_reduce
    per equal-M job group computes the min over rows into a packed strip;
    ACT applies ln/exp/relu tail (the pow, with sqrt folded into the
    exponent); strips DMA out.
  * Host: scatters strip columns into the zero-initialized canvas
    (pure placement; part of unsharding).

Uniform SPMD structure: all 8 cores run the same program; per-core values
ride in DMA'd tables; schedule shapes are max-padded across the curves
assigned to each of the two curve slots.
"""

import math
import os

import numpy as np

H, W = 256, 256
NUM_SAMPLES = 32
MAX_LENGTH = 300.0
EPSILON = 1e-6
N_CORES = 8
CURVES_PER_CORE = 2

WIN = 8           # x-window width (output cols per job)
MARGIN = 8.5      # sample relevance half-width (8 px + rounding slack)
PAD_SY = 1500.0   # pad-row sample y-offset (never wins the min)
BIG = 1.0e9


# ----------------------------------------------------------------------------
# Host-side geometry (mirrors reference.py in float64)
# ----------------------------------------------------------------------------

def _bezier_eval(cp, ts):
    K = cp.shape[0]
    n = K - 1
    i = np.arange(K)
    binom = np.array([math.comb(n, k) for k in range(K)], dtype=np.float64)
    t = ts[:, None]
    basis = binom * (t ** i) * ((1.0 - t) ** (n - i))
    return basis @ cp


def _decasteljau_left(cp, t):
    pts = cp.copy()
    left = [cp[0]]
    for _ in range(cp.shape[0] - 1):
        pts = (1.0 - t) * pts[:-1] + t * pts[1:]
        left.append(pts[0])
    return np.stack(left)


def compute_samples(inputs):
    """[B, K, 2] normalized control points -> [B, S, 2] sample points (y, x)."""
    ts = np.linspace(0.0, 1.0, NUM_SAMPLES)
    out = []
    for b in range(inputs.shape[0]):
        cp = inputs[b].astype(np.float64) * np.array([H, W], dtype=np.float64)
        approx = _bezier_eval(cp, ts)
        seg = np.diff(approx, axis=0)
        arc = np.sqrt((seg ** 2).sum(-1)).sum()
        t_tr = min(1.0, MAX_LENGTH / (arc + EPSILON))
        out.append(_bezier_eval(_decasteljau_left(cp, t_tr), ts))
    return np.stack(out)  # [B, S, 2] float64


# ----------------------------------------------------------------------------
# Planner: per-curve job lists, then a uniform padded schedule
# ----------------------------------------------------------------------------

class Job:
    __slots__ = ("xoff", "ytile", "rows")

    def __init__(self, xoff, ytile, rows):
        self.xoff = xoff      # int, window covers x in [xoff, xoff+WIN)
        self.ytile = ytile    # 0 or 1 (y in [ytile*128, ytile*128+128))
        self.rows = rows      # list of (sy, sx) float64 sample coords


def plan_curve(samples):
    """samples [S, 2] (y, x) -> list of Job."""
    sy = samples[:, 0]
    sx = samples[:, 1]
    # active x intervals (union of per-sample [sx-M, sx+M]), clipped to canvas
    lo = np.maximum(np.floor(sx - MARGIN).astype(int), 0)
    hi = np.minimum(np.ceil(sx + MARGIN).astype(int), W - 1)
    ivs = sorted((int(a), int(b)) for a, b in zip(lo, hi) if a <= b)
    merged = []
    for a, b in ivs:
        if merged and a <= merged[-1][1] + 1:
            merged[-1][1] = max(merged[-1][1], b)
        else:
            merged.append([a, b])
    # tile with WIN-wide windows
    xoffs = []
    for a, b in merged:
        o = a
        while o <= b:
            xoffs.append(o)
            o += WIN
    jobs = []
    for o in xoffs:
        # samples relevant in x
        selx = (sx + MARGIN >= o) & (sx - MARGIN < o + WIN)
        for yt in (0, 1):
            y0, y1 = yt * 128, yt * 128 + 128
            sely = (sy + MARGIN >= y0) & (sy - MARGIN < y1)
            sel = selx & sely
            if sel.any():
                rows = list(zip(sy[sel], sx[sel]))
                jobs.append(Job(o, yt, rows))
    return jobs


def round_up_M(m):
    for v in (2, 3, 4, 6, 8, 12, 16, 24, 32, 40, 48, 64):
        if m <= v:
            return v
    return m


def build_schedule(job_lists):
    """job_lists: per-curve list of Jobs (for the curves sharing one slot).
    Returns (schedule, padded) where schedule is a list of M values (one per
    job slot, descending), grouped so equal M's are adjacent."""
    per_curve = []
    for jl in job_lists:
        sizes = sorted((round_up_M(len(j.rows)) for j in jl), reverse=True)
        per_curve.append(sizes)
    nslots = max(len(s) for s in per_curve)
    sched = []
    for i in range(nslots):
        sched.append(max((s[i] if i < len(s) else 0) for s in per_curve))
    sched = [round_up_M(max(m, 2)) for m in sched]
    return sched


def group_schedule(sched):
    """[M descending] -> list of (g, M) groups of equal M."""
    groups = []
    for m in sched:
        if groups and groups[-1][1] == m:
            groups[-1][0] += 1
        else:
            groups.append([1, m])
    return [(g, m) for g, m in groups]


def pack_chunks(groups):
    """Split the (g, M) group list into chunks of <=512 T-columns so each
    chunk is one matmul into one PSUM bank, with groups never straddling a
    chunk boundary.  Returns list of chunks, each a list of (g, M)."""
    chunks = []
    cur = []
    cols = 0
    for g, M in groups:
        span = M * WIN
        assert span <= 512, (g, M)
        while g > 0:
            maxj = (512 - cols) // span
            if maxj == 0:
                chunks.append(cur)
                cur = []
                cols = 0
                maxj = 512 // span
            take = min(g, maxj)
            cur.append((take, M))
            cols += take * span
            g -= take
    if cur:
        chunks.append(cur)
    return chunks


# ----------------------------------------------------------------------------
# Table building (per curve, given its slot schedule)
# ----------------------------------------------------------------------------

def q11(x):
    """Round to 11 significant bits (safely exact under fp32r's ~12-bit
    input truncation)."""
    x = np.asarray(x, dtype=np.float64)
    m, e = np.frexp(x)
    return np.ldexp(np.round(m * 2048.0), e - 11)


def build_tables(jobs, sched, width, aa):
    """Returns (psi [6, TCOLS] f32, scatter list, tcols).

    The device computes T = phi^T @ psi in fp32r (inputs truncated to
    ~12 bits).  Every psi/phi entry is built 11-bit-exact so the matmul is
    exact up to fp32 accumulation; the sample's y is effectively displaced
    by <= ~3e-5 px (sq + srq vs sy).  Features:
      phi = [q11(y'^2), y'^2 - q11(y'^2), y', y', 1, 1]     (y' = p - 64)
      psi = [1, 1, -2*sq, -2*srq, c_hi, c_lo]
    with sq = q11(sy'), srq = q11(sy' - sq), S = sq + srq,
         c = S^2 + (x'' - sx'')^2, c_hi = q11(c), c_lo = q11(c - c_hi).
    """
    tcols = sum(m * WIN for m in sched)
    psi = np.zeros((6, tcols), dtype=np.float64)
    scatter = []
    jobs_sorted = sorted(jobs, key=lambda j: len(j.rows), reverse=True)
    assert len(jobs_sorted) <= len(sched), (len(jobs_sorted), len(sched))
    col = 0
    xl = np.arange(WIN, dtype=np.float64)
    half = WIN / 2.0
    for k, M in enumerate(sched):
        job = jobs_sorted[k] if k < len(jobs_sorted) else None
        if job is not None:
            assert round_up_M(len(job.rows)) <= M
            scatter.append((job.ytile, job.xoff))
        else:
            scatter.append(None)
        for m in range(M):
            if job is not None and m < len(job.rows):
                sy, sx = job.rows[m]
                syp = sy - (job.ytile * 128 + 64.0)   # y center
                sxp = sx - (job.xoff + half)          # window-local x center
            else:
                syp = PAD_SY
                sxp = 0.0
            sq = q11(syp)
            srq = q11(syp - sq)
            S = sq + srq
            c = S * S + (xl - half - sxp) ** 2
            c_hi = q11(c)
            c_lo = q11(c - c_hi)
            c0, c1 = col + m * WIN, col + (m + 1) * WIN
            psi[0, c0:c1] = 1.0
            psi[1, c0:c1] = 1.0
            psi[2, c0:c1] = -2.0 * sq
            psi[3, c0:c1] = -2.0 * srq
            psi[4, c0:c1] = c_hi
            psi[5, c0:c1] = c_lo
        col += M * WIN
    return psi.astype(np.float32), scatter, tcols


PHI = None


def get_phi():
    global PHI
    if PHI is None:
        p = np.arange(128, dtype=np.float64) - 64.0
        y2 = p * p
        y2hi = q11(y2)
        PHI = np.stack([y2hi, y2 - y2hi, p, p,
                        np.ones(128), np.ones(128)]).astype(np.float32)
    return PHI


# ----------------------------------------------------------------------------
# Numpy simulation of the device program (for validation)
# ----------------------------------------------------------------------------

def simulate_device(psi, groups, inv_w2, half_aa):
    """Exact-semantics simulation of one curve-slot's device pipeline.
    psi [3, TCOLS]; groups [(g, M)]; returns strip [128, J*WIN] f32."""
    phi = get_phi().astype(np.float32)
    T = (phi.T.astype(np.float32) @ psi.astype(np.float32)).astype(np.float32)
    # T [128, TCOLS]; reduce per group
    strips = []
    col = 0
    for g, M in groups:
        blk = T[:, col:col + g * M * WIN].reshape(128, g, M, WIN)
        strips.append(blk.min(axis=2).reshape(128, g * WIN))
        col += g * M * WIN
    m2 = np.concatenate(strips, axis=1).astype(np.float32)
    lnv = np.log(np.maximum(m2 * inv_w2 + 1e-12, 1e-38)).astype(np.float32)
    p = np.exp(half_aa * lnv).astype(np.float32)
    return np.maximum(1.0 - p, 0.0).astype(np.float32)


def host_scatter(canvas, strip, scatter, sched):
    """Place strip columns into canvas [256, 256] for one curve."""
    col = 0
    for k, _M in enumerate(sched):
        info = scatter[k]
        if info is not None:
            yt, xoff = info
            x0 = max(xoff, 0)
            x1 = min(xoff + WIN, W)
            if x1 > x0:
                canvas[yt * 128:(yt + 1) * 128, x0:x1] = \
                    strip[:, col + (x0 - xoff):col + (x1 - xoff)]
        col += WIN
    return canvas


# ----------------------------------------------------------------------------
# Full planning for a batch of curves
# ----------------------------------------------------------------------------

class Plan:
    pass


def plan_all(inputs, widths, aas):
    """Plan the uniform schedule + per-curve tables.
    Returns a Plan with everything needed for device build + scatter."""
    B = inputs.shape[0]
    samples = compute_samples(inputs)
    jobs = [plan_curve(samples[b]) for b in range(B)]
    # assign curves to slots: sort by max job size so similar curves share
    # a slot schedule (tighter padding)
    dens = [max((len(j.rows) for j in jl), default=0) for jl in jobs]
    order = np.argsort(dens)  # ascending density
    slot_curves = [list(order[:B // 2]), list(order[B // 2:])]
    plan = Plan()
    plan.samples = samples
    plan.slot_curves = slot_curves  # slot -> list of 8 curve ids (core i gets
    #                                  slot_curves[s][i] in slot s)
    plan.scheds = []
    plan.groups = []
    plan.chunks = []
    plan.tcols = []
    for s in range(2):
        sched = build_schedule([jobs[c] for c in slot_curves[s]])
        plan.scheds.append(sched)
        plan.groups.append(group_schedule(sched))
        plan.chunks.append(pack_chunks(plan.groups[s]))
        plan.tcols.append(sum(m * WIN for m in sched))
    plan.psis = {}      # curve -> psi
    plan.scatters = {}  # curve -> scatter list
    for s in range(2):
        for c in slot_curves[s]:
            psi, scatter, _ = build_tables(jobs[c], plan.scheds[s],
                                           widths[c], aas[c])
            plan.psis[c] = psi
            plan.scatters[c] = scatter
    plan.widths = widths
    plan.aas = aas
    return plan


# ----------------------------------------------------------------------------
# Bass device program
# ----------------------------------------------------------------------------

def build_bass(plan):
    import concourse.bacc as bacc
    import concourse.mybir as mybir
    from concourse.tile import TileContext

    dt = mybir.dt
    nc = bacc.Bacc(None, target_bir_lowering=False)

    tcols = plan.tcols
    scols = [len(plan.scheds[s]) * WIN for s in range(2)]

    phi_d = nc.dram_tensor("phi", [6, 128], dt.float32r, kind="ExternalInput")
    psi_d = [nc.dram_tensor(f"psi{s}", [6, tcols[s]], dt.float32r,
                            kind="ExternalInput") for s in range(2)]
    pars_d = nc.dram_tensor("pars", [128, 16], dt.float32,
                            kind="ExternalInput")
    strips_d = [nc.dram_tensor(f"strips{s}", [128, scols[s]], dt.float32,
                               kind="ExternalOutput") for s in range(2)]

    with TileContext(nc) as tc:
        with tc.tile_pool(name="sb", bufs=1) as pool, \
             tc.tile_pool(name="ps", bufs=1, space="PSUM") as ppool:
            # ACT table warm-up: issue a tiny Ln immediately so the
            # ~2.7us table load overlaps the input DMAs.
            warm = pool.tile([128, 8], dt.float32, tag="warm")
            nc.gpsimd.memset(warm[:], 1.0)
            warm2 = pool.tile([128, 8], dt.float32, tag="warm2")
            nc.scalar.activation(warm2[:], warm[:],
                                 mybir.ActivationFunctionType.Ln,
                                 bias=warm[:, 0:1], scale=1.0)

            # spread input DMA issue across engines (parallel descriptors)
            phit = pool.tile([6, 128], dt.float32r, tag="phi")
            nc.scalar.dma_start(out=phit[:], in_=phi_d[:])
            parst = pool.tile([128, 16], dt.float32, tag="pars")
            nc.sync.dma_start(out=parst[:], in_=pars_d[:])

            dma_in = [nc.sync, nc.gpsimd]
            psits = []
            strips = []
            tls = []
            for s in range(2):
                TC = tcols[s]
                SC = scols[s]
                psit = pool.tile([6, TC], dt.float32r, tag=f"psi{s}")
                dma_in[s].dma_start(out=psit[:], in_=psi_d[s][:])
                psits.append(psit)

                strip = pool.tile([128, SC], dt.float32, tag=f"strip{s}")
                col = 0
                sc = 0
                for chunk in plan.chunks[s]:
                    span = sum(g * M * WIN for g, M in chunk)
                    Tc = ppool.tile([128, span], dt.float32,
                                    tag=f"T{s}_{col}")
                    nc.tensor.matmul(Tc[:], phit[:],
                                     psit[:, col:col + span],
                                     start=True, stop=True)
                    ccol = 0
                    for g, M in chunk:
                        tv = Tc[:, ccol:ccol + g * M * WIN].rearrange(
                            "p (j m x) -> p j x m", j=g, m=M, x=WIN)
                        ov = strip[:, sc:sc + g * WIN].rearrange(
                            "p (j x) -> p j x", j=g, x=WIN)
                        nc.vector.tensor_reduce(out=ov, in_=tv,
                                                axis=mybir.AxisListType.X,
                                                op=mybir.AluOpType.min)
                        ccol += g * M * WIN
                        sc += g * WIN
                    col += span
                nc.vector.tensor_scalar_max(strip[:], strip[:], 0.0)
                strips.append(strip)
                tl = pool.tile([128, SC], dt.float32, tag=f"tail{s}", name=f"tail{s}")
                tls.append(tl)

            # tails batched by activation function to avoid table thrash:
            # canvas = relu(1 - exp(aa/2 * ln(max(m2,0)/w^2 + 1e-12)))
            for s in range(2):
                po = 8 * s
                nc.scalar.activation(tls[s][:], strips[s][:],
                                     mybir.ActivationFunctionType.Ln,
                                     bias=parst[:, po + 2:po + 3],
                                     scale=parst[:, po + 0:po + 1])
            for s in range(2):
                po = 8 * s
                nc.scalar.activation(tls[s][:], tls[s][:],
                                     mybir.ActivationFunctionType.Exp,
                                     bias=parst[:, po + 3:po + 4],
                                     scale=parst[:, po + 1:po + 2])
            for s in range(2):
                po = 8 * s
                nc.scalar.activation(tls[s][:], tls[s][:],
                                     mybir.ActivationFunctionType.Relu,
                                     bias=parst[:, po + 4:po + 5],
                                     scale=-1.0)
            dma_out = [nc.sync, nc.gpsimd]
            for s in range(2):
                dma_out[s].dma_start(out=strips_d[s][:], in_=tls[s][:])
    nc.compile()
    return nc


def make_pars(plan, s0_curve, s1_curve):
    pars = np.zeros((128, 16), dtype=np.float32)
    for s, c in ((0, s0_curve), (1, s1_curve)):
        po = 8 * s
        pars[:, po + 0] = 1.0 / (float(plan.widths[c]) ** 2)
        pars[:, po + 1] = float(plan.aas[c]) / 2.0
        pars[:, po + 2] = 1e-12
        pars[:, po + 3] = 0.0
        pars[:, po + 4] = 1.0
    return pars


def make_in_maps(plan):
    in_maps = []
    for core in range(N_CORES):
        c0 = int(plan.slot_curves[0][core])
        c1 = int(plan.slot_curves[1][core])
        in_maps.append({
            "phi": get_phi(),
            "psi0": plan.psis[c0],
            "psi1": plan.psis[c1],
            "pars": make_pars(plan, c0, c1),
        })
    return in_maps


def scatter_all(plan, results):
    out = np.zeros((len(plan.widths), H, W), dtype=np.float32)
    for core in range(N_CORES):
        for s in range(2):
            c = int(plan.slot_curves[s][core])
            strip = results[core][f"strips{s}"]
            host_scatter(out[c], strip, plan.scatters[c], plan.scheds[s])
    return out


def kernel(inputs, widths, aa_factors):
    inputs = np.asarray(inputs, dtype=np.float32)
    widths = np.asarray(widths, dtype=np.float32)
    aa_factors = np.asarray(aa_factors, dtype=np.float32)
    plan = plan_all(inputs, widths, aa_factors)
    nc = build_bass(plan)
    from concourse.bass_utils import run_bass_kernel_spmd
    res = run_bass_kernel_spmd(nc, make_in_maps(plan),
                               core_ids=list(range(N_CORES)))
    return scatter_all(plan, res.results)


def reference_canvas_host(plan):
    """Full-pipeline numpy simulation -> [B, 256, 256] canvases."""
    B = len(plan.widths)
    out = np.zeros((B, H, W), dtype=np.float32)
    for s in range(2):
        for c in plan.slot_curves[s]:
            inv_w2 = np.float32(1.0 / (plan.widths[c] ** 2))
            half_aa = np.float32(plan.aas[c] / 2.0)
            strip = simulate_device(plan.psis[c], plan.groups[s],
                                    inv_w2, half_aa)
            host_scatter(out[c], strip, plan.scatters[c], plan.scheds[s])
    return out


# revision 15
# speedup vs baseline: 1.2433x; 1.1745x over previous
"""Trainium2 Bass kernel for nn_CurveGraphic2d (retrieval_knn).

Computes, for B=16 cubic Bezier curves, a 256x256 canvas per curve:
    canvas = clip(1 - (min_dist_to_32_samples / w + eps)^aa, 0, 1)

Strategy (data-parallel over curves, 2 per core):
  * Host (numpy): evaluate the 32 Bezier sample points per curve (tiny),
    plan x-window "jobs": each job = (16-wide x-window, y-tile) holding M
    parabola rows (the samples relevant to that window).  Build psi tables
    so that the device matmul  phi^T @ psi  produces T[p, row, xl] =
    squared distance from pixel (y = ytile*128 + p, x = xoff + xl) to the
    row's sample.  Coordinates are centered to keep fp32r matmul exact
    enough.
  * Device: fp32r matmuls build T in PSUM; one 4-dim strided tensor_reduce
    per equal-M job group computes the min over rows into a packed strip;
    ACT applies ln/exp/relu tail (the pow, with sqrt folded into the
    exponent); strips DMA out.
  * Host: scatters strip columns into the zero-initialized canvas
    (pure placement; part of unsharding).

Uniform SPMD structure: all 8 cores run the same program; per-core values
ride in DMA'd tables; schedule shapes are max-padded across the curves
assigned to each of the two curve slots.
"""

import math
import os

import numpy as np

H, W = 256, 256
NUM_SAMPLES = 32
MAX_LENGTH = 300.0
EPSILON = 1e-6
N_CORES = 8
CURVES_PER_CORE = 2

WIN = 8           # x-window width (output cols per job)
MARGIN = 8.5      # sample relevance half-width (8 px + rounding slack)
PAD_SY = 1500.0   # pad-row sample y-offset (never wins the min)
BIG = 1.0e9


# ----------------------------------------------------------------------------
# Host-side geometry (mirrors reference.py in float64)
# ----------------------------------------------------------------------------

def _bezier_eval(cp, ts):
    K = cp.shape[0]
    n = K - 1
    i = np.arange(K)
    binom = np.array([math.comb(n, k) for k in range(K)], dtype=np.float64)
    t = ts[:, None]
    basis = binom * (t ** i) * ((1.0 - t) ** (n - i))
    return basis @ cp


def _decasteljau_left(cp, t):
    pts = cp.copy()
    left = [cp[0]]
    for _ in range(cp.shape[0] - 1):
        pts = (1.0 - t) * pts[:-1] + t * pts[1:]
        left.append(pts[0])
    return np.stack(left)


def compute_samples(inputs):
    """[B, K, 2] normalized control points -> [B, S, 2] sample points (y, x)."""
    ts = np.linspace(0.0, 1.0, NUM_SAMPLES)
    out = []
    for b in range(inputs.shape[0]):
        cp = inputs[b].astype(np.float64) * np.array([H, W], dtype=np.float64)
        approx = _bezier_eval(cp, ts)
        seg = np.diff(approx, axis=0)
        arc = np.sqrt((seg ** 2).sum(-1)).sum()
        t_tr = min(1.0, MAX_LENGTH / (arc + EPSILON))
        out.append(_bezier_eval(_decasteljau_left(cp, t_tr), ts))
    return np.stack(out)  # [B, S, 2] float64


# ----------------------------------------------------------------------------
# Planner: per-curve job lists, then a uniform padded schedule
# ----------------------------------------------------------------------------

class Job:
    __slots__ = ("xoff", "ytile", "rows")

    def __init__(self, xoff, ytile, rows):
        self.xoff = xoff      # int, window covers x in [xoff, xoff+WIN)
        self.ytile = ytile    # 0 or 1 (y in [ytile*128, ytile*128+128))
        self.rows = rows      # list of (sy, sx) float64 sample coords


def plan_curve(samples):
    """samples [S, 2] (y, x) -> list of Job."""
    sy = samples[:, 0]
    sx = samples[:, 1]
    # active x intervals (union of per-sample [sx-M, sx+M]), clipped to canvas
    lo = np.maximum(np.floor(sx - MARGIN).astype(int), 0)
    hi = np.minimum(np.ceil(sx + MARGIN).astype(int), W - 1)
    ivs = sorted((int(a), int(b)) for a, b in zip(lo, hi) if a <= b)
    merged = []
    for a, b in ivs:
        if merged and a <= merged[-1][1] + 1:
            merged[-1][1] = max(merged[-1][1], b)
        else:
            merged.append([a, b])
    # tile with WIN-wide windows
    xoffs = []
    for a, b in merged:
        o = a
        while o <= b:
            xoffs.append(o)
            o += WIN
    jobs = []
    for o in xoffs:
        # samples relevant in x
        selx = (sx + MARGIN >= o) & (sx - MARGIN < o + WIN)
        for yt in (0, 1):
            y0, y1 = yt * 128, yt * 128 + 128
            sely = (sy + MARGIN >= y0) & (sy - MARGIN < y1)
            sel = selx & sely
            if sel.any():
                rows = list(zip(sy[sel], sx[sel]))
                jobs.append(Job(o, yt, rows))
    return jobs


def round_up_M(m):
    for v in (2, 3, 4, 6, 8, 12, 16, 24, 32, 40, 48, 64):
        if m <= v:
            return v
    return m


def build_schedule(job_lists):
    """job_lists: per-curve list of Jobs (for the curves sharing one slot).
    Returns (schedule, padded) where schedule is a list of M values (one per
    job slot, descending), grouped so equal M's are adjacent."""
    per_curve = []
    for jl in job_lists:
        sizes = sorted((round_up_M(len(j.rows)) for j in jl), reverse=True)
        per_curve.append(sizes)
    nslots = max(len(s) for s in per_curve)
    sched = []
    for i in range(nslots):
        sched.append(max((s[i] if i < len(s) else 0) for s in per_curve))
    sched = [round_up_M(max(m, 2)) for m in sched]
    return sched


def group_schedule(sched):
    """[M descending] -> list of (g, M) groups of equal M."""
    groups = []
    for m in sched:
        if groups and groups[-1][1] == m:
            groups[-1][0] += 1
        else:
            groups.append([1, m])
    return [(g, m) for g, m in groups]


def pack_chunks(groups):
    """Split the (g, M) group list into chunks of <=512 T-columns so each
    chunk is one matmul into one PSUM bank, with groups never straddling a
    chunk boundary.  Returns list of chunks, each a list of (g, M)."""
    chunks = []
    cur = []
    cols = 0
    for g, M in groups:
        span = M * WIN
        assert span <= 512, (g, M)
        while g > 0:
            maxj = (512 - cols) // span
            if maxj == 0:
                chunks.append(cur)
                cur = []
                cols = 0
                maxj = 512 // span
            take = min(g, maxj)
            cur.append((take, M))
            cols += take * span
            g -= take
    if cur:
        chunks.append(cur)
    return chunks


# ----------------------------------------------------------------------------
# Table building (per curve, given its slot schedule)
# ----------------------------------------------------------------------------

def q11(x):
    """Round to 11 significant bits (safely exact under fp32r's ~12-bit
    input truncation)."""
    x = np.asarray(x, dtype=np.float64)
    m, e = np.frexp(x)
    return np.ldexp(np.round(m * 2048.0), e - 11)


def build_tables(jobs, sched, width, aa):
    """Returns (psi [6, TCOLS] f32, scatter list, tcols).

    The device computes T = phi^T @ psi in fp32r (inputs truncated to
    ~12 bits).  Every psi/phi entry is built 11-bit-exact so the matmul is
    exact up to fp32 accumulation; the sample's y is effectively displaced
    by <= ~3e-5 px (sq + srq vs sy).  Features:
      phi = [q11(y'^2), y'^2 - q11(y'^2), y', y', 1, 1]     (y' = p - 64)
      psi = [1, 1, -2*sq, -2*srq, c_hi, c_lo]
    with sq = q11(sy'), srq = q11(sy' - sq), S = sq + srq,
         c = S^2 + (x'' - sx'')^2, c_hi = q11(c), c_lo = q11(c - c_hi).
    """
    tcols = sum(m * WIN for m in sched)
    psi = np.zeros((6, tcols), dtype=np.float64)
    scatter = []
    jobs_sorted = sorted(jobs, key=lambda j: len(j.rows), reverse=True)
    assert len(jobs_sorted) <= len(sched), (len(jobs_sorted), len(sched))
    col = 0
    xl = np.arange(WIN, dtype=np.float64)
    half = WIN / 2.0
    for k, M in enumerate(sched):
        job = jobs_sorted[k] if k < len(jobs_sorted) else None
        if job is not None:
            assert round_up_M(len(job.rows)) <= M
            scatter.append((job.ytile, job.xoff))
        else:
            scatter.append(None)
        for m in range(M):
            if job is not None and m < len(job.rows):
                sy, sx = job.rows[m]
                syp = sy - (job.ytile * 128 + 64.0)   # y center
                sxp = sx - (job.xoff + half)          # window-local x center
            else:
                syp = PAD_SY
                sxp = 0.0
            sq = q11(syp)
            srq = q11(syp - sq)
            S = sq + srq
            c = S * S + (xl - half - sxp) ** 2
            c_hi = q11(c)
            c_lo = q11(c - c_hi)
            c0, c1 = col + m * WIN, col + (m + 1) * WIN
            psi[0, c0:c1] = 1.0
            psi[1, c0:c1] = 1.0
            psi[2, c0:c1] = -2.0 * sq
            psi[3, c0:c1] = -2.0 * srq
            psi[4, c0:c1] = c_hi
            psi[5, c0:c1] = c_lo
        col += M * WIN
    return psi.astype(np.float32), scatter, tcols


PHI = None


def get_phi():
    global PHI
    if PHI is None:
        p = np.arange(128, dtype=np.float64) - 64.0
        y2 = p * p
        y2hi = q11(y2)
        PHI = np.stack([y2hi, y2 - y2hi, p, p,
                        np.ones(128), np.ones(128)]).astype(np.float32)
    return PHI


# ----------------------------------------------------------------------------
# Numpy simulation of the device program (for validation)
# ----------------------------------------------------------------------------

def simulate_device(psi, groups, inv_w2, half_aa):
    """Exact-semantics simulation of one curve-slot's device pipeline.
    psi [3, TCOLS]; groups [(g, M)]; returns strip [128, J*WIN] f32."""
    phi = get_phi().astype(np.float32)
    T = (phi.T.astype(np.float32) @ psi.astype(np.float32)).astype(np.float32)
    # T [128, TCOLS]; reduce per group
    strips = []
    col = 0
    for g, M in groups:
        blk = T[:, col:col + g * M * WIN].reshape(128, g, M, WIN)
        strips.append(blk.min(axis=2).reshape(128, g * WIN))
        col += g * M * WIN
    m2 = np.concatenate(strips, axis=1).astype(np.float32)
    lnv = np.log(np.maximum(m2 * inv_w2 + 1e-12, 1e-38)).astype(np.float32)
    p = np.exp(half_aa * lnv).astype(np.float32)
    return np.maximum(1.0 - p, 0.0).astype(np.float32)


def host_scatter(canvas, strip, scatter, sched):
    """Place strip columns into canvas [256, 256] for one curve."""
    col = 0
    for k, _M in enumerate(sched):
        info = scatter[k]
        if info is not None:
            yt, xoff = info
            x0 = max(xoff, 0)
            x1 = min(xoff + WIN, W)
            if x1 > x0:
                canvas[yt * 128:(yt + 1) * 128, x0:x1] = \
                    strip[:, col + (x0 - xoff):col + (x1 - xoff)]
        col += WIN
    return canvas


# ----------------------------------------------------------------------------
# Full planning for a batch of curves
# ----------------------------------------------------------------------------

class Plan:
    pass


def plan_all(inputs, widths, aas):
    """Plan the uniform schedule + per-curve tables.
    Returns a Plan with everything needed for device build + scatter."""
    B = inputs.shape[0]
    samples = compute_samples(inputs)
    jobs = [plan_curve(samples[b]) for b in range(B)]
    # assign curves to slots: sort by max job size so similar curves share
    # a slot schedule (tighter padding)
    dens = [max((len(j.rows) for j in jl), default=0) for jl in jobs]
    order = np.argsort(dens)  # ascending density
    slot_curves = [list(order[:B // 2]), list(order[B // 2:])]
    plan = Plan()
    plan.samples = samples
    plan.slot_curves = slot_curves  # slot -> list of 8 curve ids (core i gets
    #                                  slot_curves[s][i] in slot s)
    plan.scheds = []
    plan.groups = []
    plan.chunks = []
    plan.tcols = []
    for s in range(2):
        sched = build_schedule([jobs[c] for c in slot_curves[s]])
        plan.scheds.append(sched)
        plan.groups.append(group_schedule(sched))
        plan.chunks.append(pack_chunks(plan.groups[s]))
        plan.tcols.append(sum(m * WIN for m in sched))
    plan.psis = {}      # curve -> psi
    plan.scatters = {}  # curve -> scatter list
    for s in range(2):
        for c in slot_curves[s]:
            psi, scatter, _ = build_tables(jobs[c], plan.scheds[s],
                                           widths[c], aas[c])
            plan.psis[c] = psi
            plan.scatters[c] = scatter
    plan.widths = widths
    plan.aas = aas
    return plan


# ----------------------------------------------------------------------------
# Bass device program
# ----------------------------------------------------------------------------

def build_bass(plan):
    import concourse.bacc as bacc
    import concourse.mybir as mybir
    from concourse.tile import TileContext

    dt = mybir.dt

    class _Bacc(bacc.Bacc):
        """Force Ln/Exp/Relu activations onto the single table set that
        contains all three (natural_log_exp_and_others), so the kernel pays
        exactly one ACT_TABLE_LOAD instead of thrashing between the per-
        function anchor sets."""

        def insert_act_table_loads(self):
            from concourse.hw_specs import get_activation_tables
            mine = {mybir.ActivationFunctionType.Ln,
                    mybir.ActivationFunctionType.Exp,
                    mybir.ActivationFunctionType.Relu}
            tables = []
            for name, funcs in get_activation_tables(self.m.arch).items():
                if name != "natural_log_exp_and_others":
                    funcs = funcs - mine
                tables.append((name, funcs))
            bacc._bass_rust.insert_act_table_loads(self, tables)

    nc = _Bacc(None, target_bir_lowering=False)

    tcols = plan.tcols
    scols = [len(plan.scheds[s]) * WIN for s in range(2)]

    phi_d = nc.dram_tensor("phi", [6, 128], dt.float32r, kind="ExternalInput")
    psi_d = [nc.dram_tensor(f"psi{s}", [6, tcols[s]], dt.float32r,
                            kind="ExternalInput") for s in range(2)]
    pars_d = nc.dram_tensor("pars", [128, 16], dt.float32,
                            kind="ExternalInput")
    strips_d = [nc.dram_tensor(f"strips{s}", [128, scols[s]], dt.float32,
                               kind="ExternalOutput") for s in range(2)]

    with TileContext(nc) as tc:
        with tc.tile_pool(name="sb", bufs=1) as pool, \
             tc.tile_pool(name="ps", bufs=1, space="PSUM") as ppool:
            # ACT table warm-up: issue a tiny Ln immediately so the
            # ~2.7us table load overlaps the input DMAs.
            warm = pool.tile([128, 8], dt.float32, tag="warm")
            nc.gpsimd.memset(warm[:], 1.0)
            warm2 = pool.tile([128, 8], dt.float32, tag="warm2")
            nc.scalar.activation(warm2[:], warm[:],
                                 mybir.ActivationFunctionType.Ln,
                                 bias=warm[:, 0:1], scale=1.0)

            # spread input DMA issue across engines (parallel descriptors)
            phit = pool.tile([6, 128], dt.float32r, tag="phi")
            nc.sync.dma_start(out=phit[:], in_=phi_d[:])
            parst = pool.tile([128, 16], dt.float32, tag="pars")
            nc.scalar.dma_start(out=parst[:], in_=pars_d[:])

            dma_in = [nc.sync, nc.scalar]
            psits = []
            strips = []
            tls = []
            for s in range(2):
                TC = tcols[s]
                SC = scols[s]
                psit = pool.tile([6, TC], dt.float32r, tag=f"psi{s}")
                dma_in[s].dma_start(out=psit[:], in_=psi_d[s][:])
                psits.append(psit)

                strip = pool.tile([128, SC], dt.float32, tag=f"strip{s}")
                col = 0
                sc = 0
                for chunk in plan.chunks[s]:
                    span = sum(g * M * WIN for g, M in chunk)
                    Tc = ppool.tile([128, span], dt.float32,
                                    tag=f"T{s}_{col}")
                    nc.tensor.matmul(Tc[:], phit[:],
                                     psit[:, col:col + span],
                                     start=True, stop=True)
                    ccol = 0
                    for g, M in chunk:
                        tv = Tc[:, ccol:ccol + g * M * WIN].rearrange(
                            "p (j m x) -> p j x m", j=g, m=M, x=WIN)
                        ov = strip[:, sc:sc + g * WIN].rearrange(
                            "p (j x) -> p j x", j=g, x=WIN)
                        nc.vector.tensor_reduce(out=ov, in_=tv,
                                                axis=mybir.AxisListType.X,
                                                op=mybir.AluOpType.min)
                        ccol += g * M * WIN
                        sc += g * WIN
                    col += span
                nc.vector.tensor_scalar_max(strip[:], strip[:], 0.0)
                strips.append(strip)
                tl = pool.tile([128, SC], dt.float32, tag=f"tail{s}", name=f"tail{s}")
                tls.append(tl)

            # tails batched by activation function to avoid table thrash:
            # canvas = relu(1 - exp(aa/2 * ln(max(m2,0)/w^2 + 1e-12)))
            for s in range(2):
                po = 8 * s
                nc.scalar.activation(tls[s][:], strips[s][:],
                                     mybir.ActivationFunctionType.Ln,
                                     bias=parst[:, po + 2:po + 3],
                                     scale=parst[:, po + 0:po + 1])
            for s in range(2):
                po = 8 * s
                nc.scalar.activation(tls[s][:], tls[s][:],
                                     mybir.ActivationFunctionType.Exp,
                                     bias=parst[:, po + 3:po + 4],
                                     scale=parst[:, po + 1:po + 2])
            for s in range(2):
                po = 8 * s
                nc.scalar.activation(tls[s][:], tls[s][:],
                                     mybir.ActivationFunctionType.Relu,
                                     bias=parst[:, po + 4:po + 5],
                                     scale=-1.0)
            dma_out = [nc.sync, nc.sync]
            for s in range(2):
                dma_out[s].dma_start(out=strips_d[s][:], in_=tls[s][:])
    nc.compile()
    return nc


def make_pars(plan, s0_curve, s1_curve):
    pars = np.zeros((128, 16), dtype=np.float32)
    for s, c in ((0, s0_curve), (1, s1_curve)):
        po = 8 * s
        pars[:, po + 0] = 1.0 / (float(plan.widths[c]) ** 2)
        pars[:, po + 1] = float(plan.aas[c]) / 2.0
        pars[:, po + 2] = 1e-12
        pars[:, po + 3] = 0.0
        pars[:, po + 4] = 1.0
    return pars


def make_in_maps(plan):
    in_maps = []
    for core in range(N_CORES):
        c0 = int(plan.slot_curves[0][core])
        c1 = int(plan.slot_curves[1][core])
        in_maps.append({
            "phi": get_phi(),
            "psi0": plan.psis[c0],
            "psi1": plan.psis[c1],
            "pars": make_pars(plan, c0, c1),
        })
    return in_maps


def scatter_all(plan, results):
    out = np.zeros((len(plan.widths), H, W), dtype=np.float32)
    for core in range(N_CORES):
        for s in range(2):
            c = int(plan.slot_curves[s][core])
            strip = results[core][f"strips{s}"]
            host_scatter(out[c], strip, plan.scatters[c], plan.scheds[s])
    return out


def kernel(inputs, widths, aa_factors):
    inputs = np.asarray(inputs, dtype=np.float32)
    widths = np.asarray(widths, dtype=np.float32)
    aa_factors = np.asarray(aa_factors, dtype=np.float32)
    plan = plan_all(inputs, widths, aa_factors)
    nc = build_bass(plan)
    from concourse.bass_utils import run_bass_kernel_spmd
    res = run_bass_kernel_spmd(nc, make_in_maps(plan),
                               core_ids=list(range(N_CORES)))
    return scatter_all(plan, res.results)


def reference_canvas_host(plan):
    """Full-pipeline numpy simulation -> [B, 256, 256] canvases."""
    B = len(plan.widths)
    out = np.zeros((B, H, W), dtype=np.float32)
    for s in range(2):
        for c in plan.slot_curves[s]:
            inv_w2 = np.float32(1.0 / (plan.widths[c] ** 2))
            half_aa = np.float32(plan.aas[c] / 2.0)
            strip = simulate_device(plan.psis[c], plan.groups[s],
                                    inv_w2, half_aa)
            host_scatter(out[c], strip, plan.scatters[c], plan.scheds[s])
    return out


# revision 16
# speedup vs baseline: 1.2809x; 1.0302x over previous
"""Trainium2 Bass kernel for nn_CurveGraphic2d (retrieval_knn).

Computes, for B=16 cubic Bezier curves, a 256x256 canvas per curve:
    canvas = clip(1 - (min_dist_to_32_samples / w + eps)^aa, 0, 1)

Strategy (data-parallel over curves, 2 per core):
  * Host (numpy): evaluate the 32 Bezier sample points per curve (tiny),
    plan x-window "jobs": each job = (16-wide x-window, y-tile) holding M
    parabola rows (the samples relevant to that window).  Build psi tables
    so that the device matmul  phi^T @ psi  produces T[p, row, xl] =
    squared distance from pixel (y = ytile*128 + p, x = xoff + xl) to the
    row's sample.  Coordinates are centered to keep fp32r matmul exact
    enough.
  * Device: fp32r matmuls build T in PSUM; one 4-dim strided tensor_reduce
    per equal-M job group computes the min over rows into a packed strip;
    ACT applies ln/exp/relu tail (the pow, with sqrt folded into the
    exponent); strips DMA out.
  * Host: scatters strip columns into the zero-initialized canvas
    (pure placement; part of unsharding).

Uniform SPMD structure: all 8 cores run the same program; per-core values
ride in DMA'd tables; schedule shapes are max-padded across the curves
assigned to each of the two curve slots.
"""

import math
import os

import numpy as np

H, W = 256, 256
NUM_SAMPLES = 32
MAX_LENGTH = 300.0
EPSILON = 1e-6
N_CORES = 8
CURVES_PER_CORE = 2

WIN = 8           # x-window width (output cols per job)
MARGIN = 8.5      # sample relevance half-width (8 px + rounding slack)
PAD_SY = 1500.0   # pad-row sample y-offset (never wins the min)
BIG = 1.0e9


# ----------------------------------------------------------------------------
# Host-side geometry (mirrors reference.py in float64)
# ----------------------------------------------------------------------------

def _bezier_eval(cp, ts):
    K = cp.shape[0]
    n = K - 1
    i = np.arange(K)
    binom = np.array([math.comb(n, k) for k in range(K)], dtype=np.float64)
    t = ts[:, None]
    basis = binom * (t ** i) * ((1.0 - t) ** (n - i))
    return basis @ cp


def _decasteljau_left(cp, t):
    pts = cp.copy()
    left = [cp[0]]
    for _ in range(cp.shape[0] - 1):
        pts = (1.0 - t) * pts[:-1] + t * pts[1:]
        left.append(pts[0])
    return np.stack(left)


def compute_samples(inputs):
    """[B, K, 2] normalized control points -> [B, S, 2] sample points (y, x)."""
    ts = np.linspace(0.0, 1.0, NUM_SAMPLES)
    out = []
    for b in range(inputs.shape[0]):
        cp = inputs[b].astype(np.float64) * np.array([H, W], dtype=np.float64)
        approx = _bezier_eval(cp, ts)
        seg = np.diff(approx, axis=0)
        arc = np.sqrt((seg ** 2).sum(-1)).sum()
        t_tr = min(1.0, MAX_LENGTH / (arc + EPSILON))
        out.append(_bezier_eval(_decasteljau_left(cp, t_tr), ts))
    return np.stack(out)  # [B, S, 2] float64


# ----------------------------------------------------------------------------
# Planner: per-curve job lists, then a uniform padded schedule
# ----------------------------------------------------------------------------

class Job:
    __slots__ = ("xoff", "ytile", "rows")

    def __init__(self, xoff, ytile, rows):
        self.xoff = xoff      # int, window covers x in [xoff, xoff+WIN)
        self.ytile = ytile    # 0 or 1 (y in [ytile*128, ytile*128+128))
        self.rows = rows      # list of (sy, sx) float64 sample coords


def plan_curve(samples):
    """samples [S, 2] (y, x) -> list of Job."""
    sy = samples[:, 0]
    sx = samples[:, 1]
    # active x intervals (union of per-sample [sx-M, sx+M]), clipped to canvas
    lo = np.maximum(np.floor(sx - MARGIN).astype(int), 0)
    hi = np.minimum(np.ceil(sx + MARGIN).astype(int), W - 1)
    ivs = sorted((int(a), int(b)) for a, b in zip(lo, hi) if a <= b)
    merged = []
    for a, b in ivs:
        if merged and a <= merged[-1][1] + 1:
            merged[-1][1] = max(merged[-1][1], b)
        else:
            merged.append([a, b])
    # tile with WIN-wide windows
    xoffs = []
    for a, b in merged:
        o = a
        while o <= b:
            xoffs.append(o)
            o += WIN
    jobs = []
    for o in xoffs:
        # samples relevant in x
        selx = (sx + MARGIN >= o) & (sx - MARGIN < o + WIN)
        for yt in (0, 1):
            y0, y1 = yt * 128, yt * 128 + 128
            sely = (sy + MARGIN >= y0) & (sy - MARGIN < y1)
            sel = selx & sely
            if sel.any():
                rows = list(zip(sy[sel], sx[sel]))
                jobs.append(Job(o, yt, rows))
    return jobs


def round_up_M(m):
    for v in (2, 3, 4, 6, 8, 12, 16, 24, 32, 40, 48, 64):
        if m <= v:
            return v
    return m


def build_schedule(job_lists):
    """job_lists: per-curve list of Jobs (for the curves sharing one slot).
    Returns (schedule, padded) where schedule is a list of M values (one per
    job slot, descending), grouped so equal M's are adjacent."""
    per_curve = []
    for jl in job_lists:
        sizes = sorted((round_up_M(len(j.rows)) for j in jl), reverse=True)
        per_curve.append(sizes)
    nslots = max(len(s) for s in per_curve)
    sched = []
    for i in range(nslots):
        sched.append(max((s[i] if i < len(s) else 0) for s in per_curve))
    sched = [round_up_M(max(m, 2)) for m in sched]
    return sched


def group_schedule(sched):
    """[M descending] -> list of (g, M) groups of equal M."""
    groups = []
    for m in sched:
        if groups and groups[-1][1] == m:
            groups[-1][0] += 1
        else:
            groups.append([1, m])
    return [(g, m) for g, m in groups]


def pack_chunks(groups):
    """Split the (g, M) group list into chunks of <=512 T-columns so each
    chunk is one matmul into one PSUM bank, with groups never straddling a
    chunk boundary.  Returns list of chunks, each a list of (g, M)."""
    chunks = []
    cur = []
    cols = 0
    for g, M in groups:
        span = M * WIN
        assert span <= 512, (g, M)
        while g > 0:
            maxj = (512 - cols) // span
            if maxj == 0:
                chunks.append(cur)
                cur = []
                cols = 0
                maxj = 512 // span
            take = min(g, maxj)
            cur.append((take, M))
            cols += take * span
            g -= take
    if cur:
        chunks.append(cur)
    return chunks


# ----------------------------------------------------------------------------
# Table building (per curve, given its slot schedule)
# ----------------------------------------------------------------------------

def q11(x):
    """Round to 11 significant bits (safely exact under fp32r's ~12-bit
    input truncation)."""
    x = np.asarray(x, dtype=np.float64)
    m, e = np.frexp(x)
    return np.ldexp(np.round(m * 2048.0), e - 11)


def build_tables(jobs, sched, width, aa):
    """Returns (psi [6, TCOLS] f32, scatter list, tcols).

    The device computes T = phi^T @ psi in fp32r (inputs truncated to
    ~12 bits).  Every psi/phi entry is built 11-bit-exact so the matmul is
    exact up to fp32 accumulation; the sample's y is effectively displaced
    by <= ~3e-5 px (sq + srq vs sy).  Features:
      phi = [q11(y'^2), y'^2 - q11(y'^2), y', y', 1, 1]     (y' = p - 64)
      psi = [1, 1, -2*sq, -2*srq, c_hi, c_lo]
    with sq = q11(sy'), srq = q11(sy' - sq), S = sq + srq,
         c = S^2 + (x'' - sx'')^2, c_hi = q11(c), c_lo = q11(c - c_hi).
    """
    tcols = sum(m * WIN for m in sched)
    psi = np.zeros((6, tcols), dtype=np.float64)
    scatter = []
    jobs_sorted = sorted(jobs, key=lambda j: len(j.rows), reverse=True)
    assert len(jobs_sorted) <= len(sched), (len(jobs_sorted), len(sched))
    col = 0
    xl = np.arange(WIN, dtype=np.float64)
    half = WIN / 2.0
    for k, M in enumerate(sched):
        job = jobs_sorted[k] if k < len(jobs_sorted) else None
        if job is not None:
            assert round_up_M(len(job.rows)) <= M
            scatter.append((job.ytile, job.xoff))
        else:
            scatter.append(None)
        for m in range(M):
            if job is not None and m < len(job.rows):
                sy, sx = job.rows[m]
                syp = sy - (job.ytile * 128 + 64.0)   # y center
                sxp = sx - (job.xoff + half)          # window-local x center
            else:
                syp = PAD_SY
                sxp = 0.0
            sq = q11(syp)
            srq = q11(syp - sq)
            S = sq + srq
            c = S * S + (xl - half - sxp) ** 2
            c_hi = q11(c)
            c_lo = q11(c - c_hi)
            c0, c1 = col + m * WIN, col + (m + 1) * WIN
            psi[0, c0:c1] = 1.0
            psi[1, c0:c1] = 1.0
            psi[2, c0:c1] = -2.0 * sq
            psi[3, c0:c1] = -2.0 * srq
            psi[4, c0:c1] = c_hi
            psi[5, c0:c1] = c_lo
        col += M * WIN
    return psi.astype(np.float32), scatter, tcols


PHI = None


def get_phi():
    global PHI
    if PHI is None:
        p = np.arange(128, dtype=np.float64) - 64.0
        y2 = p * p
        y2hi = q11(y2)
        PHI = np.stack([y2hi, y2 - y2hi, p, p,
                        np.ones(128), np.ones(128)]).astype(np.float32)
    return PHI


# ----------------------------------------------------------------------------
# Numpy simulation of the device program (for validation)
# ----------------------------------------------------------------------------

def simulate_device(psi, groups, inv_w2, half_aa):
    """Exact-semantics simulation of one curve-slot's device pipeline.
    psi [3, TCOLS]; groups [(g, M)]; returns strip [128, J*WIN] f32."""
    phi = get_phi().astype(np.float32)
    T = (phi.T.astype(np.float32) @ psi.astype(np.float32)).astype(np.float32)
    # T [128, TCOLS]; reduce per group
    strips = []
    col = 0
    for g, M in groups:
        blk = T[:, col:col + g * M * WIN].reshape(128, g, M, WIN)
        strips.append(blk.min(axis=2).reshape(128, g * WIN))
        col += g * M * WIN
    m2 = np.concatenate(strips, axis=1).astype(np.float32)
    lnv = np.log(np.maximum(m2 * inv_w2 + 1e-12, 1e-38)).astype(np.float32)
    p = np.exp(half_aa * lnv).astype(np.float32)
    return np.maximum(1.0 - p, 0.0).astype(np.float32)


def host_scatter(canvas, strip, scatter, sched):
    """Place strip columns into canvas [256, 256] for one curve."""
    col = 0
    for k, _M in enumerate(sched):
        info = scatter[k]
        if info is not None:
            yt, xoff = info
            x0 = max(xoff, 0)
            x1 = min(xoff + WIN, W)
            if x1 > x0:
                canvas[yt * 128:(yt + 1) * 128, x0:x1] = \
                    strip[:, col + (x0 - xoff):col + (x1 - xoff)]
        col += WIN
    return canvas


# ----------------------------------------------------------------------------
# Full planning for a batch of curves
# ----------------------------------------------------------------------------

class Plan:
    pass


def plan_all(inputs, widths, aas):
    """Plan the uniform schedule + per-curve tables.
    Returns a Plan with everything needed for device build + scatter."""
    B = inputs.shape[0]
    samples = compute_samples(inputs)
    jobs = [plan_curve(samples[b]) for b in range(B)]
    # assign curves to slots: sort by max job size so similar curves share
    # a slot schedule (tighter padding)
    dens = [max((len(j.rows) for j in jl), default=0) for jl in jobs]
    order = np.argsort(dens)  # ascending density
    slot_curves = [list(order[:B // 2]), list(order[B // 2:])]
    plan = Plan()
    plan.samples = samples
    plan.slot_curves = slot_curves  # slot -> list of 8 curve ids (core i gets
    #                                  slot_curves[s][i] in slot s)
    plan.scheds = []
    plan.groups = []
    plan.chunks = []
    plan.tcols = []
    for s in range(2):
        sched = build_schedule([jobs[c] for c in slot_curves[s]])
        plan.scheds.append(sched)
        plan.groups.append(group_schedule(sched))
        plan.chunks.append(pack_chunks(plan.groups[s]))
        plan.tcols.append(sum(m * WIN for m in sched))
    plan.psis = {}      # curve -> psi
    plan.scatters = {}  # curve -> scatter list
    for s in range(2):
        for c in slot_curves[s]:
            psi, scatter, _ = build_tables(jobs[c], plan.scheds[s],
                                           widths[c], aas[c])
            plan.psis[c] = psi
            plan.scatters[c] = scatter
    plan.widths = widths
    plan.aas = aas
    return plan


# ----------------------------------------------------------------------------
# Bass device program
# ----------------------------------------------------------------------------

def build_bass(plan):
    import concourse.bacc as bacc
    import concourse.mybir as mybir
    from concourse.tile import TileContext

    dt = mybir.dt

    class _Bacc(bacc.Bacc):
        """Force Ln/Exp/Relu activations onto the single table set that
        contains all three (natural_log_exp_and_others), so the kernel pays
        exactly one ACT_TABLE_LOAD instead of thrashing between the per-
        function anchor sets."""

        def insert_act_table_loads(self):
            from concourse.hw_specs import get_activation_tables
            mine = {mybir.ActivationFunctionType.Ln,
                    mybir.ActivationFunctionType.Exp,
                    mybir.ActivationFunctionType.Relu}
            tables = []
            for name, funcs in get_activation_tables(self.m.arch).items():
                if name != "natural_log_exp_and_others":
                    funcs = funcs - mine
                tables.append((name, funcs))
            bacc._bass_rust.insert_act_table_loads(self, tables)

    nc = _Bacc(None, target_bir_lowering=False)

    tcols = plan.tcols
    scols = [len(plan.scheds[s]) * WIN for s in range(2)]

    phi_d = nc.dram_tensor("phi", [6, 128], dt.float32r, kind="ExternalInput")
    psi_d = [nc.dram_tensor(f"psi{s}", [6, tcols[s]], dt.float32r,
                            kind="ExternalInput") for s in range(2)]
    pars_d = nc.dram_tensor("pars", [128, 16], dt.float32,
                            kind="ExternalInput")
    strips_d = [nc.dram_tensor(f"strips{s}", [128, scols[s]], dt.float32,
                               kind="ExternalOutput") for s in range(2)]

    with TileContext(nc) as tc:
        with tc.tile_pool(name="sb", bufs=1) as pool, \
             tc.tile_pool(name="ps", bufs=1, space="PSUM") as ppool:
            # ACT table warm-up: issue a tiny Ln immediately so the
            # ~2.7us table load overlaps the input DMAs.
            warm = pool.tile([128, 8], dt.float32, tag="warm")
            nc.gpsimd.memset(warm[:], 1.0)
            warm2 = pool.tile([128, 8], dt.float32, tag="warm2")
            nc.scalar.activation(warm2[:], warm[:],
                                 mybir.ActivationFunctionType.Ln,
                                 bias=warm[:, 0:1], scale=1.0)

            # spread input DMA issue across engines (parallel descriptors)
            phit = pool.tile([6, 128], dt.float32r, tag="phi")
            nc.sync.dma_start(out=phit[:], in_=phi_d[:])
            parst = pool.tile([128, 16], dt.float32, tag="pars")
            nc.scalar.dma_start(out=parst[:], in_=pars_d[:])

            dma_eng = [nc.sync, nc.scalar]
            strips = []
            tls = []
            di = 0
            for s in range(2):
                SC = scols[s]
                strip = pool.tile([128, SC], dt.float32, tag=f"strip{s}",
                                  name=f"strip{s}")
                col = 0
                sc = 0
                for ci, chunk in enumerate(plan.chunks[s]):
                    span = sum(g * M * WIN for g, M in chunk)
                    psic = pool.tile([6, span], dt.float32r,
                                     tag=f"psi{s}_{ci}", name=f"psi{s}_{ci}")
                    dma_eng[di % 2].dma_start(
                        out=psic[:], in_=psi_d[s][:, col:col + span])
                    di += 1
                    Tc = ppool.tile([128, span], dt.float32,
                                    tag=f"T{s}_{ci}", name=f"T{s}_{ci}")
                    nc.tensor.matmul(Tc[:], phit[:], psic[:],
                                     start=True, stop=True)
                    ccol = 0
                    for g, M in chunk:
                        tv = Tc[:, ccol:ccol + g * M * WIN].rearrange(
                            "p (j m x) -> p j x m", j=g, m=M, x=WIN)
                        ov = strip[:, sc:sc + g * WIN].rearrange(
                            "p (j x) -> p j x", j=g, x=WIN)
                        nc.vector.tensor_reduce(out=ov, in_=tv,
                                                axis=mybir.AxisListType.X,
                                                op=mybir.AluOpType.min)
                        ccol += g * M * WIN
                        sc += g * WIN
                    col += span
                nc.vector.tensor_scalar_max(strip[:], strip[:], 0.0)
                strips.append(strip)
                tl = pool.tile([128, SC], dt.float32, tag=f"tail{s}", name=f"tail{s}")
                tls.append(tl)

            # tails batched by activation function to avoid table thrash:
            # canvas = relu(1 - exp(aa/2 * ln(max(m2,0)/w^2 + 1e-12)))
            for s in range(2):
                po = 8 * s
                nc.scalar.activation(tls[s][:], strips[s][:],
                                     mybir.ActivationFunctionType.Ln,
                                     bias=parst[:, po + 2:po + 3],
                                     scale=parst[:, po + 0:po + 1])
            for s in range(2):
                po = 8 * s
                nc.scalar.activation(tls[s][:], tls[s][:],
                                     mybir.ActivationFunctionType.Exp,
                                     bias=parst[:, po + 3:po + 4],
                                     scale=parst[:, po + 1:po + 2])
            for s in range(2):
                po = 8 * s
                nc.scalar.activation(tls[s][:], tls[s][:],
                                     mybir.ActivationFunctionType.Relu,
                                     bias=parst[:, po + 4:po + 5],
                                     scale=-1.0)
            dma_out = [nc.sync, nc.sync]
            for s in range(2):
                dma_out[s].dma_start(out=strips_d[s][:], in_=tls[s][:])
    nc.compile()
    return nc


def make_pars(plan, s0_curve, s1_curve):
    pars = np.zeros((128, 16), dtype=np.float32)
    for s, c in ((0, s0_curve), (1, s1_curve)):
        po = 8 * s
        pars[:, po + 0] = 1.0 / (float(plan.widths[c]) ** 2)
        pars[:, po + 1] = float(plan.aas[c]) / 2.0
        pars[:, po + 2] = 1e-12
        pars[:, po + 3] = 0.0
        pars[:, po + 4] = 1.0
    return pars


def make_in_maps(plan):
    in_maps = []
    for core in range(N_CORES):
        c0 = int(plan.slot_curves[0][core])
        c1 = int(plan.slot_curves[1][core])
        in_maps.append({
            "phi": get_phi(),
            "psi0": plan.psis[c0],
            "psi1": plan.psis[c1],
            "pars": make_pars(plan, c0, c1),
        })
    return in_maps


def scatter_all(plan, results):
    out = np.zeros((len(plan.widths), H, W), dtype=np.float32)
    for core in range(N_CORES):
        for s in range(2):
            c = int(plan.slot_curves[s][core])
            strip = results[core][f"strips{s}"]
            host_scatter(out[c], strip, plan.scatters[c], plan.scheds[s])
    return out


def kernel(inputs, widths, aa_factors):
    inputs = np.asarray(inputs, dtype=np.float32)
    widths = np.asarray(widths, dtype=np.float32)
    aa_factors = np.asarray(aa_factors, dtype=np.float32)
    plan = plan_all(inputs, widths, aa_factors)
    nc = build_bass(plan)
    from concourse.bass_utils import run_bass_kernel_spmd
    res = run_bass_kernel_spmd(nc, make_in_maps(plan),
                               core_ids=list(range(N_CORES)))
    return scatter_all(plan, res.results)


def reference_canvas_host(plan):
    """Full-pipeline numpy simulation -> [B, 256, 256] canvases."""
    B = len(plan.widths)
    out = np.zeros((B, H, W), dtype=np.float32)
    for s in range(2):
        for c in plan.slot_curves[s]:
            inv_w2 = np.float32(1.0 / (plan.widths[c] ** 2))
            half_aa = np.float32(plan.aas[c] / 2.0)
            strip = simulate_device(plan.psis[c], plan.groups[s],
                                    inv_w2, half_aa)
            host_scatter(out[c], strip, plan.scatters[c], plan.scheds[s])
    return out
